# revision 1
# baseline (speedup 1.0000x reference)
"""MoE layer (top-2 of 8 experts) on 8 trn2 NeuronCores, expert-parallel.

Strategy (per the expert-parallel sharding hint):
  - Host computes the tiny gating network (x @ Wg + bg, softmax, top-2) in
    float64 numpy — 0.27 GFLOP of the 137 GFLOP total — and dispatches
    tokens by expert id: core e receives the tokens routed to expert e,
    pre-scaled by their gate weight and laid out transposed for the PE.
  - Each core runs a pure tiled matmul  y_e = xt_e.T @ We[e]  over its
    [C, 2048] packed token block in float32r (1 cycle/row on the PE vs 4
    for plain fp32).  Weights for expert e are loaded by exactly one core.
  - Host scatters the per-expert outputs back: slot-0 rows are a plain
    assignment (they partition the token set), slot-1 rows are an add.
    The be bias term (w0*be[e0] + w1*be[e1] per token) is added on host.

This does 4x less matmul work than dense dispatch (top-2 of 8 experts) and
is compute-bound: ~2176*2048*2048*2 = 18.3 GFLOP per core.
"""

import numpy as np

N_CORES = 8
N, D, H, E = 8192, 2048, 2048, 8
TOP_K = 2
KT = D // 128  # 16 contraction tiles
HT = H // 512  # 4 output column chunks
C_DEFAULT = 2176  # per-expert token capacity (17 * 128); balanced routing
                  # of 2*8192/8 = 2048 avg tokens/expert leaves ~6% slack

_program_cache: dict[tuple[int, int], object] = {}


def build_program(C: int, repeat: int = 1):
    """SPMD program for one core: y[C, H] = xt[., ., ., .].T @ we[D, H].

    xt layout is [C//128, 128, KT, 128] with xt[m, p, k, j] holding
    (w * x[token])[128*m + j, 128*k + p] so that each m-tile is one
    contiguous 1 MB DMA and xt[m][:, k, :] is directly the [K=128, M=128]
    stationary operand of the PE matmul.
    """
    import concourse.tile as tile
    from concourse import bacc, mybir

    f32 = mybir.dt.float32
    f32r = mybir.dt.float32r
    MT = C // 128

    nc = bacc.Bacc("TRN2", target_bir_lowering=False, debug=False,
                   num_devices=N_CORES)
    xt = nc.declare_dram_parameter("xt", [MT, 128, KT, 128], f32r,
                                   isOutput=False)
    we = nc.declare_dram_parameter("we", [D, H], f32r, isOutput=False)
    y = nc.declare_dram_parameter("y", [C, H], f32, isOutput=True)

    with tile.TileContext(nc) as tc:
        with (
            tc.tile_pool(name="wp", bufs=1) as wp,
            tc.tile_pool(name="xp", bufs=3) as xp,
            tc.tile_pool(name="op", bufs=3) as op,
            tc.tile_pool(name="ps", bufs=4, space="PSUM") as ps,
        ):
            # Expert weights, fully resident: 64 [128, 512] tiles keyed by
            # (k, h), loaded h-major so the first psum group only waits on
            # the 16 (k, h=0) tiles (4.2 MB) instead of all 16.8 MB.
            we_sb = {}
            for h in range(HT):
                for k in range(KT):
                    t = wp.tile([128, 512], f32r, tag=f"we{k}_{h}")
                    nc.sync.dma_start(
                        t[:], we[128 * k:128 * (k + 1), 512 * h:512 * (h + 1)]
                    )
                    we_sb[(k, h)] = t

            for r in range(repeat):
                for m in range(MT):
                    xt_m = xp.tile([128, KT, 128], f32r, tag="xt")
                    nc.sync.dma_start(xt_m[:], xt[m])
                    out_m = op.tile([128, H], f32, tag="out")
                    for h in range(HT):
                        acc = ps.tile([128, 512], f32, tag="acc")
                        for k in range(KT):
                            nc.tensor.matmul(
                                acc[:], xt_m[:, k, :], we_sb[(k, h)][:],
                                start=(k == 0), stop=(k == KT - 1),
                            )
                        nc.vector.tensor_copy(
                            out_m[:, 512 * h:512 * (h + 1)], acc[:]
                        )
                    nc.sync.dma_start(y[128 * m:128 * (m + 1), :], out_m[:])
    nc.compile()
    return nc


def _get_program(C: int, repeat: int = 1):
    key = (C, repeat)
    if key not in _program_cache:
        _program_cache[key] = build_program(C, repeat)
    return _program_cache[key]


def route(x, Wg, bg):
    """Gating + top-2 routing on host (float64 for a stable ordering).

    Returns (e0, e1, w0, w1): per-token top-1/top-2 expert ids and their
    (unnormalized) softmax gate weights, matching jax.lax.top_k tie-break
    (lower index wins).
    """
    logits = x.astype(np.float64) @ Wg.astype(np.float64) + bg.astype(np.float64)
    order = np.argsort(-logits, axis=1, kind="stable")
    e0 = order[:, 0].astype(np.int32)
    e1 = order[:, 1].astype(np.int32)
    mx = logits.max(axis=1, keepdims=True)
    p = np.exp(logits - mx)
    gate = p / p.sum(axis=1, keepdims=True)
    n = np.arange(logits.shape[0])
    w0 = gate[n, e0].astype(np.float32)
    w1 = gate[n, e1].astype(np.float32)
    return e0, e1, w0, w1


def kernel(x, Wg, bg, We, be):
    x = np.ascontiguousarray(np.asarray(x, dtype=np.float32))
    Wg = np.asarray(Wg, dtype=np.float32)
    bg = np.asarray(bg, dtype=np.float32)
    We = np.asarray(We, dtype=np.float32)
    be = np.asarray(be, dtype=np.float32)

    e0, e1, w0, w1 = route(x, Wg, bg)

    # Per-expert token lists: slot-0 tokens first, then slot-1 tokens.
    idx0 = [np.nonzero(e0 == e)[0] for e in range(E)]
    idx1 = [np.nonzero(e1 == e)[0] for e in range(E)]
    counts = [len(idx0[e]) + len(idx1[e]) for e in range(E)]
    cmax = max(counts)
    C = max(C_DEFAULT, ((cmax + 127) // 128) * 128)

    nc = _get_program(C)

    in_maps = []
    for e in range(E):
        idx = np.concatenate([idx0[e], idx1[e]])
        w = np.concatenate([w0[idx0[e]], w1[idx1[e]]])
        xq = np.zeros((C, D), dtype=np.float32)
        xq[:len(idx)] = x[idx] * w[:, None]
        # [C, D] -> [MT, 128, KT, 128] with axes (m, p, k, j)
        a = xq.reshape(C // 128, 128, KT, 128).transpose(0, 3, 2, 1)
        in_maps.append({
            "xt": np.ascontiguousarray(a),
            "we": np.ascontiguousarray(We[e]),
        })

    from concourse.bass_utils import run_bass_kernel_spmd
    res = run_bass_kernel_spmd(nc, in_maps, core_ids=list(range(N_CORES)))

    out = np.empty((N, H), dtype=np.float32)
    for e in range(E):
        y = res.results[e]["y"]
        n0 = len(idx0[e])
        out[idx0[e]] = y[:n0]
    for e in range(E):
        y = res.results[e]["y"]
        n0 = len(idx0[e])
        out[idx1[e]] += y[n0:counts[e]]

    if be.any():
        out += w0[:, None] * be[e0] + w1[:, None] * be[e1]
    return out


# revision 4
# speedup vs baseline: 16.3065x; 16.3065x over previous
"""MoE layer (top-2 of 8 experts) on 8 trn2 NeuronCores, expert-parallel.

Strategy (per the expert-parallel sharding hint):
  - Host computes the tiny gating network (x @ Wg + bg, softmax, top-2) in
    float64 numpy — 0.27 GFLOP of the 137 GFLOP total — and dispatches
    tokens by expert id: core e receives the tokens routed to expert e,
    pre-scaled by their gate weight and laid out transposed for the PE.
  - Each core runs a pure tiled matmul  y_e = xt_e.T @ We[e]  over its
    [C, 2048] packed token block in float32r (1 cycle/row on the PE vs 4
    for plain fp32).  Weights for expert e are loaded by exactly one core.
  - Host scatters the per-expert outputs back: slot-0 rows are a plain
    assignment (they partition the token set), slot-1 rows are an add.
    The be bias term (w0*be[e0] + w1*be[e1] per token) is added on host.

This does 4x less matmul work than dense dispatch (top-2 of 8 experts) and
is compute-bound: ~2176*2048*2048*2 = 18.3 GFLOP per core.
"""

import numpy as np

N_CORES = 8
N, D, H, E = 8192, 2048, 2048, 8
TOP_K = 2
KT = D // 128  # 16 contraction tiles
HT = H // 512  # 4 output column chunks
C_DEFAULT = 2176  # per-expert token capacity (17 * 128); balanced routing
                  # of 2*8192/8 = 2048 avg tokens/expert leaves ~6% slack

_program_cache: dict[tuple[int, int], object] = {}


def build_program(C: int, repeat: int = 1, loop_repeat: int = 1):
    """SPMD program for one core: y[C, H] = xt[., ., ., .].T @ we[D, H].

    xt layout is [C//128, 128, KT, 128] with xt[m, p, k, j] holding
    (w * x[token])[128*m + j, 128*k + p] so that each m-tile is one
    contiguous 1 MB DMA and xt[m][:, k, :] is directly the [K=128, M=128]
    stationary operand of the PE matmul.
    """
    import concourse.tile as tile
    from concourse import bacc, mybir

    f32 = mybir.dt.float32
    f32r = mybir.dt.float32r
    MT = C // 128

    nc = bacc.Bacc("TRN2", target_bir_lowering=False, debug=False,
                   num_devices=N_CORES)
    xt = nc.declare_dram_parameter("xt", [MT, 128, KT, 128], f32r,
                                   isOutput=False)
    we = nc.declare_dram_parameter("we", [D, H], f32r, isOutput=False)
    y = nc.declare_dram_parameter("y", [C, H], f32, isOutput=True)

    with tile.TileContext(nc) as tc:
        with (
            tc.tile_pool(name="wp", bufs=1) as wp,
            tc.tile_pool(name="xp", bufs=3) as xp,
            tc.tile_pool(name="op", bufs=3) as op,
            tc.tile_pool(name="ps", bufs=4, space="PSUM") as ps,
        ):
            # Expert weights, fully resident: 64 [128, 512] tiles keyed by
            # (k, h), loaded h-major so the first psum group only waits on
            # the 16 (k, h=0) tiles (4.2 MB) instead of all 16.8 MB.
            we_sb = {}
            for h in range(HT):
                for k in range(KT):
                    t = wp.tile([128, 512], f32r, tag=f"we{k}_{h}")
                    nc.sync.dma_start(
                        t[:], we[128 * k:128 * (k + 1), 512 * h:512 * (h + 1)]
                    )
                    we_sb[(k, h)] = t

            def body():
                for m in range(MT):
                    xt_m = xp.tile([128, KT, 128], f32r, tag="xt")
                    nc.sync.dma_start(xt_m[:], xt[m])
                    out_m = op.tile([128, H], f32, tag="out")
                    for h in range(HT):
                        acc = ps.tile([128, 512], f32, tag="acc")
                        for k in range(KT):
                            nc.tensor.matmul(
                                acc[:], xt_m[:, k, :], we_sb[(k, h)][:],
                                start=(k == 0), stop=(k == KT - 1),
                            )
                        nc.vector.tensor_copy(
                            out_m[:, 512 * h:512 * (h + 1)], acc[:]
                        )
                    nc.sync.dma_start(y[128 * m:128 * (m + 1), :], out_m[:])

            if loop_repeat > 1:
                from concourse import mybir as _mb
                with tc.For_i(0, loop_repeat, 1,
                              hint_engines=(_mb.EngineType.PE,)):
                    for _ in range(repeat):
                        body()
            else:
                for _ in range(repeat):
                    body()
    nc.compile()
    return nc


def _get_program(C: int, repeat: int = 1, loop_repeat: int = 1):
    key = (C, repeat, loop_repeat)
    if key not in _program_cache:
        _program_cache[key] = build_program(C, repeat, loop_repeat)
    return _program_cache[key]


def route(x, Wg, bg):
    """Gating + top-2 routing on host (float64 for a stable ordering).

    Returns (e0, e1, w0, w1): per-token top-1/top-2 expert ids and their
    (unnormalized) softmax gate weights, matching jax.lax.top_k tie-break
    (lower index wins).
    """
    logits = x.astype(np.float64) @ Wg.astype(np.float64) + bg.astype(np.float64)
    order = np.argsort(-logits, axis=1, kind="stable")
    e0 = order[:, 0].astype(np.int32)
    e1 = order[:, 1].astype(np.int32)
    mx = logits.max(axis=1, keepdims=True)
    p = np.exp(logits - mx)
    gate = p / p.sum(axis=1, keepdims=True)
    n = np.arange(logits.shape[0])
    w0 = gate[n, e0].astype(np.float32)
    w1 = gate[n, e1].astype(np.float32)
    return e0, e1, w0, w1


def kernel(x, Wg, bg, We, be):
    x = np.ascontiguousarray(np.asarray(x, dtype=np.float32))
    Wg = np.asarray(Wg, dtype=np.float32)
    bg = np.asarray(bg, dtype=np.float32)
    We = np.asarray(We, dtype=np.float32)
    be = np.asarray(be, dtype=np.float32)

    e0, e1, w0, w1 = route(x, Wg, bg)

    # Per-expert token lists: slot-0 tokens first, then slot-1 tokens.
    idx0 = [np.nonzero(e0 == e)[0] for e in range(E)]
    idx1 = [np.nonzero(e1 == e)[0] for e in range(E)]
    counts = [len(idx0[e]) + len(idx1[e]) for e in range(E)]
    cmax = max(counts)
    C = max(C_DEFAULT, ((cmax + 127) // 128) * 128)

    nc = _get_program(C)

    in_maps = []
    for e in range(E):
        idx = np.concatenate([idx0[e], idx1[e]])
        w = np.concatenate([w0[idx0[e]], w1[idx1[e]]])
        xq = np.zeros((C, D), dtype=np.float32)
        xq[:len(idx)] = x[idx] * w[:, None]
        # [C, D] -> [MT, 128, KT, 128] with axes (m, p, k, j)
        a = xq.reshape(C // 128, 128, KT, 128).transpose(0, 3, 2, 1)
        in_maps.append({
            "xt": np.ascontiguousarray(a),
            "we": np.ascontiguousarray(We[e]),
        })

    from concourse.bass_utils import run_bass_kernel_spmd
    res = run_bass_kernel_spmd(nc, in_maps, core_ids=list(range(N_CORES)))

    out = np.empty((N, H), dtype=np.float32)
    for e in range(E):
        y = res.results[e]["y"]
        n0 = len(idx0[e])
        out[idx0[e]] = y[:n0]
    for e in range(E):
        y = res.results[e]["y"]
        n0 = len(idx0[e])
        out[idx1[e]] += y[n0:counts[e]]

    if be.any():
        out += w0[:, None] * be[e0] + w1[:, None] * be[e1]
    return out
